# revision 33
# baseline (speedup 1.0000x reference)
"""Trainium2 Bass kernel for nn_DSTFPE (dual-stream transformer, linear attention).

Data-parallel over batch: 8 NeuronCores, one batch element each.
Layout: feature-major activations [E partitions (k-tiles of 128), S free].
Stream state fp32r; attention internals + FFN expanded parts bf16; psum fp32.
"""
import sys

sys.path.insert(0, "/opt/trn_rl_repo")

import numpy as np
import ml_dtypes
from contextlib import ExitStack

import concourse.bass as bass
import concourse.tile as tile
from concourse import mybir, bacc

F32R = mybir.dt.float32r
F32 = mybir.dt.float32
BF16 = mybir.dt.bfloat16
AF = mybir.ActivationFunctionType
OP = mybir.AluOpType

E = 512
H = 8
D = 64
RANK = 256
EXP = 1536
OUT = 15
NC = 128
SCH = 512

BF = ml_dtypes.bfloat16


def _col(v):
    v = np.asarray(v, np.float32)
    w = v.shape[0]
    n = (w + NC - 1) // NC
    pad = np.zeros((n * NC,), np.float32)
    pad[:w] = v
    return np.ascontiguousarray(pad.reshape(n, NC).T)


def _row(v, width=E, dtype=np.float32):
    v = np.asarray(v, np.float32)
    r = np.zeros((1, width), np.float32)
    r[0, : v.shape[0]] = v
    return r.astype(dtype)


def pack_params(params, NL):
    """name -> np.ndarray for DRAM params. Names starting with '_' are host-only."""
    out = {}

    def attn(pfx, p):
        out[pfx + "qd"] = np.asarray(p["qd"], np.float32).astype(BF)
        out[pfx + "kd"] = np.asarray(p["kd"], np.float32).astype(BF)
        out[pfx + "vd"] = np.asarray(p["vd"], np.float32).astype(BF)
        out[pfx + "qu"] = np.asarray(p["qu_w"], np.float32).astype(BF)
        out[pfx + "ku"] = np.asarray(p["ku_w"], np.float32).astype(BF)
        out[pfx + "vu"] = np.asarray(p["vu_w"], np.float32).astype(BF)
        out[pfx + "qu_b"] = _col(p["qu_b"])
        out[pfx + "ku_b"] = _row(p["ku_b"], E, BF)
        out[pfx + "vu_b"] = _row(p["vu_b"], E, BF)
        out[pfx + "ow"] = np.asarray(p["out_w"], np.float32).astype(BF)
        out[pfx + "ow_b"] = _col(p["out_b"])

    def ffn(pfx, p):
        g = np.asarray(p["bn_g"], np.float32)
        b = np.asarray(p["bn_b"], np.float32)
        m = np.asarray(p["bn_m"], np.float32)
        v = np.asarray(p["bn_v"], np.float32)
        s = g / np.sqrt(v + 1e-5)
        dwf = np.asarray(p["dw"], np.float32)[:, 0, :] * s[:, None]
        dwb = (np.asarray(p["dw_b"], np.float32) - m) * s + b
        out[pfx + "w1"] = np.asarray(p["w1"], np.float32).astype(BF)
        out[pfx + "b1"] = _col(p["b1"])
        out[pfx + "w2"] = np.asarray(p["w2"], np.float32).astype(BF)
        out[pfx + "b2"] = _col(p["b2"])
        out[pfx + "dw0"] = _col(dwf[:, 0])
        out[pfx + "dw1"] = _col(dwf[:, 1])
        out[pfx + "dw2"] = _col(dwf[:, 2])
        out[pfx + "dwb"] = _col(dwb)
        out[pfx + "w3"] = np.asarray(p["w3"], np.float32).astype(BF)
        out[pfx + "b3"] = _col(p["b3"])
        out[pfx + "w4"] = np.asarray(p["w4"], np.float32).astype(BF)
        out[pfx + "b4"] = _row(p["b4"], E)

    for i, p in enumerate(params["layers"][:NL]):
        pf = f"L{i}_"
        attn(pf + "sab_", p["sa_b"])
        attn(pf + "sal_", p["sa_l"])
        attn(pf + "cbl_", p["ca_b2l"])
        attn(pf + "clb_", p["ca_l2b"])
        ffn(pf + "fb_", p["ffn_b"])
        ffn(pf + "fl_", p["ffn_l"])
        out[pf + "g1"] = np.asarray(p["g1"], np.float32).astype(BF)
        out[pf + "g1b"] = _col(p["g1b"])
        out[pf + "g2"] = np.asarray(p["g2"], np.float32).astype(BF)
        out[pf + "g2b"] = _col(p["g2b"])
        g3 = np.asarray(p["g3"], np.float32)
        out[pf + "g3d"] = (g3[:, 0:1] - g3[:, 1:2]).astype(BF)
        g3b = np.asarray(p["g3b"], np.float32)
        out["_" + pf + "g3bd"] = float(g3b[0] - g3b[1])
        for nm in ["n1b", "n1l", "n2b", "n2l", "n3b", "n3l"]:
            out[pf + nm + "_g"] = _col(p[nm + "_g"])
            out[pf + nm + "_b"] = _col(p[nm + "_b"])

    fp = params["fusion"]
    out["fu_w1"] = np.asarray(fp["w1"], np.float32).astype(BF)
    out["fu_b1"] = _col(fp["b1"])
    out["fu_w2"] = np.asarray(fp["w2"], np.float32).astype(BF)
    out["fu_b2"] = _col(fp["b2"])
    out["fu_w3"] = np.asarray(fp["w3"], np.float32).astype(BF)
    out["fu_b3"] = _row(fp["b3"], E)
    out["fu_g"] = _col(fp["ln_g"])
    out["fu_bb"] = _col(fp["ln_b"])
    rp = params["reg"]
    out["rg_w1"] = np.asarray(rp["w1"], np.float32).astype(BF)
    out["rg_b1"] = _col(rp["b1"])
    out["rg_w2"] = np.asarray(rp["w2"], np.float32).astype(BF)
    out["rg_b2"] = _col(rp["b2"])
    out["rg_w3"] = np.asarray(rp["w3"], np.float32).astype(BF)
    out["rg_b3"] = _col(rp["b3"])
    out["rg_w4"] = np.asarray(rp["w4"], np.float32).astype(BF)
    out["rg_b4"] = _col(rp["b4"])
    out["ones_row"] = np.ones((1, E), np.float32)
    blk = np.zeros((2, NC), np.float32)
    blk[0, 0:D] = 1.0
    blk[1, D:2 * D] = 1.0
    out["blkones"] = blk.astype(BF)
    sel = np.zeros((4, 4 * NC), np.float32)
    for j in range(4):
        sel[j, j * NC:(j + 1) * NC] = 1.0
    out["sel4"] = sel
    return out


class Prog:
    def __init__(self, S, NL):
        self.S = S
        self.NL = NL
        self.NJ = S // SCH
        self.NI = S // NC
        self.dram = {}
        self.nc = bacc.Bacc()
        self.rr = 0
        self.rr_ps = 0
        import os
        self.dbg_stage = os.environ.get("KDBG", "")

    def dparam(self, name, shape, dtype):
        h = self.nc.declare_dram_parameter(name, list(shape), dtype, isOutput=False)
        self.dram[name] = h
        return h

    def veng(self, i):
        return self.nc.vector if i % 2 == 0 else self.nc.gpsimd


def load_w(P, wpool, name, dtype, tag="w", bufs=2):
    """Weight [K, M] -> SBUF [kp, ktiles, M] (kp = min(K,128))."""
    h = P.dram[name]
    K, M = h.shape
    kp = min(K, NC)
    kt = (K + kp - 1) // kp
    t = wpool.tile([kp, kt, M], dtype, tag=tag, bufs=bufs)
    if kt == 1:
        P.nc.sync.dma_start(t[:, 0, :], h.ap())
    else:
        P.nc.sync.dma_start(t[:, :, :], h.ap().rearrange("(t p) m -> p t m", p=kp))
    return t


def load_col(P, pool, name):
    h = P.dram[name]
    t = pool.tile([NC, h.shape[1]], F32, tag="cols", bufs=14)
    P.nc.sync.dma_start(t[:, :], h.ap())
    return t


def load_row(P, pool, name, dtype):
    h = P.dram[name]
    t = pool.tile([1, h.shape[1]], dtype, tag="rows", bufs=4)
    P.nc.sync.dma_start(t[:, :], h.ap())
    return t


def psum_t(P, shape=None, tag="mm"):
    P.rr_ps += 1
    bufs = 4 if tag == "mm" else 1
    return P.pools["psum"].tile(shape or [NC, SCH], F32, tag=tag, bufs=bufs,
                                name=f"ps{P.rr_ps}")


def proj_A(P, W, x, kt, mt, evac, extra_mm=None, x2=None, kt2=0):
    nc = P.nc
    for m in range(mt):
        for j in range(P.NJ):
            ps = psum_t(P)
            tot = kt + kt2
            for k in range(kt):
                nc.tensor.matmul(
                    ps[:, :], W[:, k, m * NC:(m + 1) * NC],
                    x[:, k, j * SCH:(j + 1) * SCH],
                    start=(k == 0), stop=(k == tot - 1 and extra_mm is None))
            for k in range(kt2):
                nc.tensor.matmul(
                    ps[:, :], W[:, kt + k, m * NC:(m + 1) * NC],
                    x2[:, k, j * SCH:(j + 1) * SCH],
                    start=False, stop=(kt + k == tot - 1 and extra_mm is None))
            if extra_mm is not None:
                extra_mm(ps, m, j)
            evac(ps, m, j)


def proj_B(P, x, W, kt, evac, bias_row=None, ones_row=None, n_out=E):
    nc = P.nc
    for i in range(P.NI):
        ps = psum_t(P, [NC, n_out])
        for k in range(kt):
            nc.tensor.matmul(
                ps[:, :], x[:, k, i * NC:(i + 1) * NC], W[:, k, 0:n_out],
                start=(k == 0), stop=(k == kt - 1 and bias_row is None))
        if bias_row is not None:
            nc.tensor.matmul(ps[:, :], ones_row[:, 0:NC], bias_row[:, 0:n_out],
                             start=False, stop=True)
        evac(ps, i)


def emit_linattn(P, tc, cols_pool, pfx, xq, xkv, dst, resid):
    """Chunk-streaming linear attention. All [*, S] temporaries are per-chunk."""
    nc = P.nc
    S, NJ, NI = P.S, P.NJ, P.NI
    C = P.consts
    ap = tc.alloc_tile_pool(name="attnA", bufs=1)
    w = dict(
        qd=load_w(P, ap, pfx + "qd", BF16, tag="wqd", bufs=1),
        kd=load_w(P, ap, pfx + "kd", BF16, tag="wkd", bufs=1),
        vd=load_w(P, ap, pfx + "vd", BF16, tag="wvd", bufs=1),
        qu=load_w(P, ap, pfx + "qu", BF16, tag="wqu", bufs=1),
        ku=load_w(P, ap, pfx + "ku", BF16, tag="wku", bufs=1),
        vu=load_w(P, ap, pfx + "vu", BF16, tag="wvu", bufs=1),
        qu_b=load_col(P, cols_pool, pfx + "qu_b"),
        ku_b=load_row(P, cols_pool, pfx + "ku_b", BF16),
        vu_b=load_row(P, cols_pool, pfx + "vu_b", BF16),
        ow=load_w(P, ap, pfx + "ow", BF16, tag="wow", bufs=1),
        ow_b=load_col(P, cols_pool, pfx + "ow_b"),
    )
    qb = w["qu_b"]
    ow_b = w["ow_b"]
    NIC = SCH // NC  # s-tiles per chunk

    # ---- pass 1: k, v chunks -> kv + ksum accumulation ----
    kv_ps = psum_t(P, [NC, 4, D], tag="kvacc")
    ks_ps = psum_t(P, [1, E], tag="ksacc")
    for j in range(NJ):
        jc = slice(j * SCH, (j + 1) * SCH)
        kdc = ap.tile([NC, 2, SCH], BF16, tag="kdc", bufs=2)
        vdc = ap.tile([NC, 2, SCH], BF16, tag="vdc", bufs=2)
        for (wd, dc) in ((w["kd"], kdc), (w["vd"], vdc)):
            for m in range(2):
                ps = psum_t(P)
                for k in range(4):
                    nc.tensor.matmul(ps[:, :], wd[:, k, m * NC:(m + 1) * NC],
                                     xkv[:, k, jc], start=(k == 0), stop=(k == 3))
                if (P.rr % 2) == 0:
                    nc.scalar.activation(dc[:, m, :], ps[:, :], AF.Copy)
                else:
                    nc.vector.tensor_copy(dc[:, m, :], ps[:, :])
                P.rr += 1
        k_sm = ap.tile([NC, NIC, E], BF16, tag="k_smc", bufs=2)
        v_sm = ap.tile([NC, NIC, E], BF16, tag="v_smc", bufs=2)
        for i in range(NIC):
            ps = psum_t(P, [NC, E])
            for k in range(2):
                nc.tensor.matmul(ps[:, :], kdc[:, k, i * NC:(i + 1) * NC],
                                 w["ku"][:, k, 0:E], start=(k == 0), stop=False)
            nc.tensor.matmul(ps[:, :], C["ones_row_bf"][:, 0:NC], w["ku_b"][:, 0:E],
                             start=False, stop=True)
            mneg = ap.tile([NC, E], BF16, tag="elu_m", bufs=2)
            nc.vector.tensor_scalar(mneg[:, :], ps[:, :], 0.0, None, op0=OP.min)
            ex = ap.tile([NC, E], BF16, tag="elu_e", bufs=2)
            nc.scalar.activation(ex[:, :], mneg[:, :], AF.Exp)
            nc.vector.scalar_tensor_tensor(k_sm[:, i, :], ps[:, :], 0.0, ex[:, :],
                                           op0=OP.max, op1=OP.add)
        for i in range(NIC):
            ps = psum_t(P, [NC, E])
            for k in range(2):
                nc.tensor.matmul(ps[:, :], vdc[:, k, i * NC:(i + 1) * NC],
                                 w["vu"][:, k, 0:E], start=(k == 0), stop=False)
            nc.tensor.matmul(ps[:, :], C["ones_row_bf"][:, 0:NC], w["vu_b"][:, 0:E],
                             start=False, stop=True)
            nc.scalar.activation(v_sm[:, i, :], ps[:, :], AF.Copy)
        last = (j == NJ - 1)
        for i in range(NIC):
            gi = j * NIC + i
            nc.tensor.matmul(ks_ps[:, :], C["ones_col_bf"][:, 0:1], k_sm[:, i, :],
                             start=(gi == 0), stop=(gi == NI - 1))
            for t in range(4):
                for hh in range(2):
                    h = 2 * t + hh
                    nc.tensor.matmul(
                        kv_ps[hh * D:(hh + 1) * D, t, :],
                        k_sm[:, i, h * D:(h + 1) * D], v_sm[:, i, h * D:(h + 1) * D],
                        start=(gi == 0), stop=(gi == NI - 1), skip_group_check=True,
                        tile_position=(0, hh * D))
    kv_sb = ap.tile([NC, 4, D], BF16, tag="kv_sb")
    nc.vector.tensor_copy(kv_sb[:, :, :], kv_ps[:, :, :])
    ks_row = ap.tile([1, E], BF16, tag="ks_row")
    nc.vector.tensor_copy(ks_row[:, :], ks_ps[:, :])
    kc_ps = psum_t(P, [NC, 4])
    for t in range(4):
        nc.tensor.matmul(kc_ps[:, t:t + 1], ks_row[0:1, t * NC:(t + 1) * NC],
                         C["ones_col_bf"][0:1, 0:1], start=True, stop=True,
                         skip_group_check=True)
    ksb = ap.tile([NC, 4, 2], BF16, tag="ksb")
    nc.vector.memset(ksb[:, :, :], 0.0)
    for t in range(4):
        nc.vector.tensor_copy(ksb[0:D, t, 0:1], kc_ps[0:D, t:t + 1])
        nc.vector.tensor_copy(ksb[D:NC, t, 1:2], kc_ps[D:NC, t:t + 1])

    # ---- pass 2: q chunks -> den -> scaled q -> att -> out ----
    for j in range(NJ):
        jc = slice(j * SCH, (j + 1) * SCH)
        qdc = ap.tile([NC, 2, SCH], BF16, tag="qdc", bufs=2)
        for m in range(2):
            ps = psum_t(P)
            for k in range(4):
                nc.tensor.matmul(ps[:, :], w["qd"][:, k, m * NC:(m + 1) * NC],
                                 xq[:, k, jc], start=(k == 0), stop=(k == 3))
            if (P.rr % 2) == 0:
                nc.scalar.activation(qdc[:, m, :], ps[:, :], AF.Copy)
            else:
                nc.vector.tensor_copy(qdc[:, m, :], ps[:, :])
            P.rr += 1
        q = ap.tile([NC, 4, SCH], BF16, tag="qc", bufs=2)
        for m in range(4):
            ps = psum_t(P)
            for k in range(2):
                nc.tensor.matmul(ps[:, :], w["qu"][:, k, m * NC:(m + 1) * NC],
                                 qdc[:, k, :], start=(k == 0), stop=(k == 1))
            mneg = ap.tile([NC, SCH], BF16, tag="elu_m", bufs=2)
            nc.vector.tensor_scalar(mneg[:, :], ps[:, :], qb[:, m:m + 1], 0.0,
                                    op0=OP.add, op1=OP.min)
            ex = ap.tile([NC, SCH], BF16, tag="elu_e", bufs=2)
            nc.scalar.activation(ex[:, :], mneg[:, :], AF.Exp)
            pos = ap.tile([NC, SCH], BF16, tag="elu_p", bufs=2)
            nc.scalar.activation(pos[:, :], ps[:, :], AF.Relu, bias=qb[:, m:m + 1])
            P.veng(m + j).tensor_tensor(q[:, m, :], ex[:, :], pos[:, :], op=OP.add)
        rden = ap.tile([2, 4, SCH], BF16, tag="rden", bufs=2)
        for t in range(4):
            dps = psum_t(P, [2, SCH])
            nc.tensor.matmul(dps[0:2, :], ksb[:, t, :], q[:, t, :],
                             start=True, stop=True, skip_group_check=True)
            dsb = ap.tile([2, SCH], F32, tag="dsb", bufs=2)
            nc.vector.tensor_scalar(dsb[:, :], dps[:, :], 1e-6, None, op0=OP.add)
            with nc.allow_low_precision(reason="rden bf16 by design"):
                nc.vector.reciprocal(rden[:, t, :], dsb[:, :])
        att = ap.tile([NC, 4, SCH], BF16, tag="attc", bufs=2)
        for t in range(4):
            bc = psum_t(P)
            nc.tensor.matmul(bc[:, :], C["blkones_bf"][:, 0:NC], rden[0:2, t, :],
                             start=True, stop=True)
            sqt = ap.tile([NC, SCH], BF16, tag="sqt", bufs=2)
            nc.vector.scalar_tensor_tensor(sqt[:, :], q[:, t, :], 1.0, bc[:, :],
                                           op0=OP.mult, op1=OP.mult)
            ps = psum_t(P)
            for hh in range(2):
                nc.tensor.matmul(
                    ps[hh * D:(hh + 1) * D, :], kv_sb[hh * D:(hh + 1) * D, t, :],
                    sqt[hh * D:(hh + 1) * D, :],
                    start=True, stop=True, skip_group_check=True,
                    tile_position=(hh * D, hh * D))
            if (P.rr % 2) == 0:
                nc.scalar.activation(att[:, t, :], ps[:, :], AF.Copy)
            else:
                nc.vector.tensor_copy(att[:, t, :], ps[:, :])
            P.rr += 1
        for m in range(4):
            ps = psum_t(P)
            for k in range(4):
                nc.tensor.matmul(ps[:, :], w["ow"][:, k, m * NC:(m + 1) * NC],
                                 att[:, k, :], start=(k == 0), stop=(k == 3))
            sl = (slice(None), m, jc)
            if resid is not None:
                nc.vector.scalar_tensor_tensor(dst[sl], ps[:, :], ow_b[:, m:m + 1],
                                               resid[sl], op0=OP.add, op1=OP.add)
            else:
                nc.scalar.activation(dst[sl], ps[:, :], AF.Identity,
                                     bias=ow_b[:, m:m + 1])
    ap.release()


def emit_ln(P, y, gcol, bcol, dst, relu=False):
    """dst = [relu](unitLN(y) * g + b), feature-dim LN. y bf16, dst fp32r."""
    nc = P.nc
    NJ = P.NJ
    lp = P.pools["ln"]
    C = P.consts

    st_s = lp.tile([NJ, SCH], F32, tag="st_s", bufs=1)
    st_q = lp.tile([NJ, SCH], F32, tag="st_q", bufs=1)
    for j in range(NJ):
        st_ps = psum_t(P, [1, 2, SCH], tag="big")
        for t in range(4):
            y2 = lp.tile([NC, SCH], BF16, tag="y2", bufs=2)
            nc.scalar.activation(y2[:, :], y[:, t, j * SCH:(j + 1) * SCH], AF.Square)
            nc.tensor.matmul(st_ps[0:1, 0, :], C["ones_col_bf"][:, 0:1],
                             y[:, t, j * SCH:(j + 1) * SCH],
                             start=(t == 0), stop=(t == 3), skip_group_check=True)
            nc.tensor.matmul(st_ps[0:1, 1, :], C["ones_col_bf"][:, 0:1], y2[:, :],
                             start=(t == 0), stop=(t == 3), skip_group_check=True)
        st_flat = lp.tile([1, 2, SCH], F32, tag="st_flat", bufs=1)
        nc.scalar.activation(st_flat[:, :, :], st_ps[:, :, :], AF.Copy)
        nc.sync.dma_start(st_s[j:j + 1, :], st_flat[0:1, 0, :])
        nc.sync.dma_start(st_q[j:j + 1, :], st_flat[0:1, 1, :])
    m_sb = lp.tile([NJ, SCH], F32R, tag="m_sb", bufs=1)
    nc.vector.tensor_scalar(m_sb[:, :], st_s[:, :], 1.0 / E, None, op0=OP.mult)
    ex2 = lp.tile([NJ, SCH], F32, tag="ex2", bufs=1)
    nc.vector.tensor_scalar(ex2[:, :], st_q[:, :], 1.0 / E, None, op0=OP.mult)
    msq = lp.tile([NJ, SCH], F32, tag="msq", bufs=1)
    nc.scalar.activation(msq[:, :], m_sb[:, :], AF.Square)
    var = lp.tile([NJ, SCH], F32, tag="var", bufs=1)
    nc.vector.tensor_tensor(var[:, :], ex2[:, :], msq[:, :], op=OP.subtract)
    lnv = lp.tile([NJ, SCH], F32, tag="lnv", bufs=1)
    nc.scalar.activation(lnv[:, :], var[:, :], AF.Ln, bias=C["eps_col"][0:NJ, 0:1])
    rstd = lp.tile([NJ, SCH], F32R, tag="rstd", bufs=1)
    nc.scalar.activation(rstd[:, :], lnv[:, :], AF.Exp, scale=-0.5)
    # broadcast mean and rstd
    mb = lp.tile([NC, NJ, SCH], BF16, tag="mb", bufs=1)
    ab = lp.tile([NC, NJ, SCH], BF16, tag="ab", bufs=1)
    for j in range(NJ):
        bcm = psum_t(P)
        nc.tensor.matmul(bcm[:, :], C["sel4"][0:NJ, j * NC:(j + 1) * NC], m_sb[:, :],
                         start=True, stop=True)
        nc.scalar.activation(mb[:, j, :], bcm[:, :], AF.Copy)
        bca = psum_t(P)
        nc.tensor.matmul(bca[:, :], C["sel4"][0:NJ, j * NC:(j + 1) * NC], rstd[:, :],
                         start=True, stop=True)
        nc.scalar.activation(ab[:, j, :], bca[:, :], AF.Copy)
    # apply
    for t in range(4):
        for j in range(NJ):
            sl = (slice(None), t, slice(j * SCH, (j + 1) * SCH))
            t1 = lp.tile([NC, SCH], BF16, tag="t1", bufs=2)
            P.veng(t + j).tensor_tensor(t1[:, :], y[sl], mb[:, j, :],
                                        op=OP.subtract)
            t2 = lp.tile([NC, SCH], BF16, tag="t2", bufs=2)
            nc.vector.scalar_tensor_tensor(t2[:, :], t1[:, :], gcol[:, t:t + 1],
                                           ab[:, j, :], op0=OP.mult, op1=OP.mult)
            nc.scalar.activation(dst[sl], t2[:, :],
                                 AF.Relu if relu else AF.Identity,
                                 bias=bcol[:, t:t + 1])


def emit_ffn(P, w, x, dst):
    """dst = x + ffn(x). x fp32r [128,4,S], dst bf16 [128,4,S]."""
    nc = P.nc
    NJ = P.NJ
    fp = P.pools["ffn"]
    C = P.consts

    def relu6_evac(dst_ap, bias_ap, eng_i):
        def f(ps):
            t6 = fp.tile(list(ps.shape), BF16, tag="t6", bufs=2)
            nc.scalar.activation(t6[:, :], ps[:, :], AF.Relu, bias=bias_ap)
            nc.vector.tensor_scalar(dst_ap, t6[:, :], 6.0, None, op0=OP.min)
        return f

    def make_h2(jj):
        h1t = fp.tile([NC, 6, SCH], BF16, tag="h1", bufs=1)
        for m in range(6):
            ps = psum_t(P)
            for k in range(4):
                nc.tensor.matmul(ps[:, :], w["w1"][:, k, m * NC:(m + 1) * NC],
                                 x[:, k, jj * SCH:(jj + 1) * SCH],
                                 start=(k == 0), stop=(k == 3))
            relu6_evac(h1t[:, m, :], w["b1"][:, m:m + 1], m)(ps)
        h2t = fp.tile([NC, 12, SCH], BF16, tag="h2", bufs=2)
        for m in range(12):
            ps = psum_t(P)
            for k in range(6):
                nc.tensor.matmul(ps[:, :], w["w2"][:, k, m * NC:(m + 1) * NC],
                                 h1t[:, k, :], start=(k == 0), stop=(k == 5))
            relu6_evac(h2t[:, m, :], w["b2"][:, m:m + 1], m)(ps)
        edge = fp.tile([NC, 12, 1], BF16, tag="edge", bufs=2)
        nc.vector.tensor_copy(edge[:, :, :], h2t[:, :, SCH - 1:SCH])
        return h2t, edge

    def conv_chunk(left, cur, right, cv):
        for t in range(12):
            d0 = w["dw0"][:, t:t + 1]
            d1 = w["dw1"][:, t:t + 1]
            d2 = w["dw2"][:, t:t + 1]
            db = w["dwb"][:, t:t + 1]
            t1 = fp.tile([NC, SCH], BF16, tag="cv1", bufs=2)
            nc.vector.tensor_scalar(t1[:, :], cur[:, t, :], d1, db,
                                    op0=OP.mult, op1=OP.add)
            u = fp.tile([NC, SCH], BF16, tag="cv2", bufs=2)
            nc.vector.scalar_tensor_tensor(u[:, 1:SCH], cur[:, t, 0:SCH - 1], d0,
                                           t1[:, 1:SCH], op0=OP.mult, op1=OP.add)
            if left is not None:
                nc.vector.scalar_tensor_tensor(u[:, 0:1], left[:, t, 0:1], d0,
                                               t1[:, 0:1], op0=OP.mult, op1=OP.add)
            else:
                nc.vector.tensor_copy(u[:, 0:1], t1[:, 0:1])
            vout = fp.tile([NC, SCH], BF16, tag="cv3", bufs=2)
            nc.vector.scalar_tensor_tensor(vout[:, 0:SCH - 1], cur[:, t, 1:SCH],
                                           d2, u[:, 0:SCH - 1],
                                           op0=OP.mult, op1=OP.add)
            if right is not None:
                nc.vector.scalar_tensor_tensor(vout[:, SCH - 1:SCH], right[:, t, 0:1],
                                               d2, u[:, SCH - 1:SCH],
                                               op0=OP.mult, op1=OP.add)
            else:
                nc.vector.tensor_copy(vout[:, SCH - 1:SCH], u[:, SCH - 1:SCH])
            nc.vector.tensor_scalar(cv[:, t, :], vout[:, :], 0.0, 6.0,
                                    op0=OP.max, op1=OP.min)

    def w3_w4_chunk(jj, cv):
        h3 = fp.tile([NC, 6, SCH], BF16, tag="h3", bufs=1)
        for m in range(6):
            ps = psum_t(P)
            for k in range(12):
                nc.tensor.matmul(ps[:, :], w["w3"][:, k, m * NC:(m + 1) * NC],
                                 cv[:, k, :], start=(k == 0), stop=(k == 11))
            relu6_evac(h3[:, m, :], w["b3"][:, m:m + 1], m)(ps)
        for m in range(4):
            ps = psum_t(P)
            for k in range(6):
                nc.tensor.matmul(ps[:, :], w["w4"][:, k, m * NC:(m + 1) * NC],
                                 h3[:, k, :], start=(k == 0), stop=False)
            nc.tensor.matmul(ps[:, :], w["b4"][:, m * NC:(m + 1) * NC],
                             C["ones_row"][:, 0:SCH], start=False, stop=True)
            sl = (slice(None), m, slice(jj * SCH, (jj + 1) * SCH))
            nc.vector.scalar_tensor_tensor(dst[sl], ps[:, :], 0.0, x[sl],
                                           op0=OP.add, op1=OP.add)

    ring = []
    for jj in range(NJ):
        ring.append(make_h2(jj))
        if jj >= 1:
            cv = fp.tile([NC, 12, SCH], BF16, tag="cv", bufs=1)
            conv_chunk(ring[jj - 2][1] if jj >= 2 else None, ring[jj - 1][0],
                       ring[jj][0], cv)
            w3_w4_chunk(jj - 1, cv)
    cv = fp.tile([NC, 12, SCH], BF16, tag="cv", bufs=1)
    conv_chunk(ring[NJ - 2][1] if NJ >= 2 else None, ring[NJ - 1][0], None, cv)
    w3_w4_chunk(NJ - 1, cv)


def emit_gate_and_mix(P, w, g3bd, cb, cl, b1, l1, yb, yl):
    nc = P.nc
    NJ = P.NJ
    gp = P.pools["gate"]
    C = P.consts

    g1o = gp.tile([NC, 2, P.S], BF16, tag="g1o")

    def evac_g1(ps, m, j):
        t6 = gp.tile([NC, SCH], BF16, tag="g1t", bufs=3)
        nc.scalar.activation(t6[:, :], ps[:, :], AF.Relu, bias=w["g1b"][:, m:m + 1])
        nc.vector.tensor_scalar(g1o[:, m, j * SCH:(j + 1) * SCH], t6[:, :],
                                    6.0, None, op0=OP.min)

    proj_A(P, w["g1"], cb, 4, 2, evac_g1, x2=cl, kt2=4)

    g2o = gp.tile([NC, 1, P.S], BF16, tag="g2o")

    def evac_g2(ps, m, j):
        t6 = gp.tile([NC, SCH], BF16, tag="g2t", bufs=3)
        nc.scalar.activation(t6[:, :], ps[:, :], AF.Relu, bias=w["g2b"][:, 0:1])
        nc.vector.tensor_scalar(g2o[:, 0, j * SCH:(j + 1) * SCH], t6[:, :],
                                6.0, None, op0=OP.min)

    proj_A(P, w["g2"], g1o, 2, 1, evac_g2)

    # d = g2o @ g3d ; g0 = sigmoid(d + g3bd); g1v = 1 - g0
    gb = gp.tile([1, 1], F32, tag="gb")
    nc.vector.memset(gb[:, :], -g3bd)
    eneg = gp.tile([1, NJ, SCH], F32, tag="eneg")
    for j in range(NJ):
        dps = psum_t(P, [1, SCH])
        nc.tensor.matmul(dps[:, :], w["g3d"][:, 0, 0:1],
                         g2o[:, 0, j * SCH:(j + 1) * SCH], start=True, stop=True,
                         skip_group_check=True)
        nc.scalar.activation(eneg[:, j, :], dps[:, :], AF.Exp, scale=-1.0,
                             bias=gb[0:1, 0:1])
    den = gp.tile([1, NJ, SCH], F32, tag="gden")
    nc.vector.tensor_scalar(den[:, :, :], eneg[:, :, :], 1.0, None, op0=OP.add)
    g0 = gp.tile([1, NJ, SCH], F32R, tag="g0")
    with nc.allow_low_precision(reason="f32r gate"):
        nc.vector.reciprocal(g0[:, :, :], den[:, :, :])
    g1v = gp.tile([1, NJ, SCH], F32R, tag="g1v")
    nc.vector.tensor_tensor(g1v[:, :, :], eneg[:, :, :], g0[:, :, :], op=OP.mult)

    for (gv, cx, base, yy) in ((g0, cb, b1, yb), (g1v, cl, l1, yl)):
        for j in range(NJ):
            bc = psum_t(P)
            nc.tensor.matmul(bc[:, :], C["ones_row"][:, 0:NC], gv[0:1, j, :],
                             start=True, stop=True)
            for t in range(4):
                sl = (slice(None), t, slice(j * SCH, (j + 1) * SCH))
                tmp = gp.tile([NC, SCH], BF16, tag="gtmp", bufs=3)
                nc.vector.scalar_tensor_tensor(tmp[:, :], cx[sl], 1.0, bc[:, :],
                                               op0=OP.mult, op1=OP.mult)
                P.veng(t + j).tensor_tensor(yy[sl], tmp[:, :], base[sl], op=OP.add)


def dump_dbg(P, t):
    tmp = P.pools["dbgpool"].tile([NC, 4, P.S], F32, tag="dbgt")
    P.nc.vector.tensor_copy(tmp[:, :, :], t[:, :, :])
    P.nc.sync.dma_start(P.dbg_d.ap(), tmp[:, :, :])


def build_program(S, NL, packed):
    P = Prog(S, NL)
    nc = P.nc
    for name, arr in packed.items():
        if name.startswith("_"):
            continue
        if arr.dtype == BF:
            dt = BF16
        elif arr.ndim == 2 and arr.shape[0] == NC and arr.dtype == np.float32:
            dt = F32  # per-partition scalar/bias columns
        else:
            dt = F32R
        P.dparam(name, arr.shape, dt)
    body_d = P.dparam("body", (4, NC, S), BF16)
    limb_d = P.dparam("limb", (4, NC, S), BF16)
    out_d = nc.declare_dram_parameter("out", [OUT, S], F32, isOutput=True)
    P.dbg_d = (nc.declare_dram_parameter("dbg", [NC, 4, S], F32, isOutput=True)
               if P.dbg_stage else None)

    with tile.TileContext(nc) as tc, ExitStack() as ctx:
        consts_pool = ctx.enter_context(tc.tile_pool(name="consts", bufs=1))
        state_pool = ctx.enter_context(tc.tile_pool(name="state", bufs=1))
        cols_pool = ctx.enter_context(tc.tile_pool(name="cols", bufs=1))
        psum_pool = ctx.enter_context(tc.tile_pool(name="psum", bufs=1, space="PSUM"))
        P.pools = {"psum": psum_pool}
        P.pools["dbgpool"] = ctx.enter_context(tc.tile_pool(name="dbgp", bufs=1)) if P.dbg_stage else None

        ones_row = consts_pool.tile([1, E], F32R)
        nc.sync.dma_start(ones_row[:, :], P.dram["ones_row"].ap())
        ones_row_bf = consts_pool.tile([1, E], BF16)
        nc.vector.tensor_copy(ones_row_bf[:, :], ones_row[:, :])
        ones_col_bf = consts_pool.tile([NC, 1], BF16)
        nc.vector.memset(ones_col_bf[:, :], 1.0)
        blkones_bf = consts_pool.tile([2, NC], BF16)
        nc.sync.dma_start(blkones_bf[:, :], P.dram["blkones"].ap())
        sel4 = consts_pool.tile([4, 4 * NC], F32R)
        nc.sync.dma_start(sel4[:, :], P.dram["sel4"].ap())
        eps_col = consts_pool.tile([NC, 1], F32)
        nc.vector.memset(eps_col[:, :], 1e-5)
        P.consts = dict(ones_row=ones_row, ones_row_bf=ones_row_bf,
                        ones_col_bf=ones_col_bf, blkones_bf=blkones_bf,
                        eps_col=eps_col, sel4=sel4)

        b_cur = state_pool.tile([NC, 4, S], BF16)
        l_cur = state_pool.tile([NC, 4, S], BF16)
        nc.sync.dma_start(b_cur[:, :, :], body_d.ap().rearrange("t p s -> p t s"))
        nc.sync.dma_start(l_cur[:, :, :], limb_d.ap().rearrange("t p s -> p t s"))

        for li in range(NL):
            pf = f"L{li}_"
            with tc.tile_pool(name=f"layer{li}", bufs=1) as layer_pool:
                P.pools["ln"] = layer_pool
                ph = tc.alloc_tile_pool(name="ph1b", bufs=1)
                yb = ph.tile([NC, 4, S], BF16, tag="yb")
                emit_linattn(P, tc, cols_pool, pf + "sab_", b_cur, b_cur, yb, b_cur)
                emit_ln(P, yb, load_col(P, cols_pool, pf + "n1b_g"),
                        load_col(P, cols_pool, pf + "n1b_b"), b_cur)
                ph.release()
                ph = tc.alloc_tile_pool(name="ph1l", bufs=1)
                yl = ph.tile([NC, 4, S], BF16, tag="yl")
                emit_linattn(P, tc, cols_pool, pf + "sal_", l_cur, l_cur, yl, l_cur)
                emit_ln(P, yl, load_col(P, cols_pool, pf + "n1l_g"),
                        load_col(P, cols_pool, pf + "n1l_b"), l_cur)
                ph.release()
                cross_pool = tc.alloc_tile_pool(name="cross", bufs=1)
                cb = cross_pool.tile([NC, 4, S], BF16, tag="cb")
                emit_linattn(P, tc, cols_pool, pf + "cbl_", b_cur, l_cur, cb, None)
                cl = cross_pool.tile([NC, 4, S], BF16, tag="cl")
                emit_linattn(P, tc, cols_pool, pf + "clb_", l_cur, b_cur, cl, None)
                gy = tc.alloc_tile_pool(name="gatey", bufs=1)
                yb = gy.tile([NC, 4, S], BF16, tag="yb2")
                yl = gy.tile([NC, 4, S], BF16, tag="yl2")
                with tc.tile_pool(name="gate", bufs=1) as gate_pool:
                    P.pools["gate"] = gate_pool
                    gw = dict(
                        g1=load_w(P, gate_pool, pf + "g1", BF16, tag="wg1", bufs=1),
                        g1b=load_col(P, cols_pool, pf + "g1b"),
                        g2=load_w(P, gate_pool, pf + "g2", BF16, tag="wg2", bufs=1),
                        g2b=load_col(P, cols_pool, pf + "g2b"),
                        g3d=load_w(P, gate_pool, pf + "g3d", BF16, tag="wg3", bufs=1),
                    )
                    emit_gate_and_mix(P, gw, packed["_" + pf + "g3bd"],
                                      cb, cl, b_cur, l_cur, yb, yl)
                emit_ln(P, yb, load_col(P, cols_pool, pf + "n2b_g"),
                        load_col(P, cols_pool, pf + "n2b_b"), b_cur)
                emit_ln(P, yl, load_col(P, cols_pool, pf + "n2l_g"),
                        load_col(P, cols_pool, pf + "n2l_b"), l_cur)
                gy.release()
                cross_pool.release()
                for (pfx, xcur, nm) in ((pf + "fb_", b_cur, "n3b"), (pf + "fl_", l_cur, "n3l")):
                    ph = tc.alloc_tile_pool(name="phf", bufs=1)
                    yy = ph.tile([NC, 4, S], BF16, tag="yf")
                    with tc.tile_pool(name="ffn", bufs=1) as ffn_pool:
                        P.pools["ffn"] = ffn_pool
                        fw = dict(
                            w1=load_w(P, ffn_pool, pfx + "w1", BF16, tag="ww1", bufs=1),
                            b1=load_col(P, cols_pool, pfx + "b1"),
                            w2=load_w(P, ffn_pool, pfx + "w2", BF16, tag="ww2", bufs=1),
                            b2=load_col(P, cols_pool, pfx + "b2"),
                            dw0=load_col(P, cols_pool, pfx + "dw0"),
                            dw1=load_col(P, cols_pool, pfx + "dw1"),
                            dw2=load_col(P, cols_pool, pfx + "dw2"),
                            dwb=load_col(P, cols_pool, pfx + "dwb"),
                            w3=load_w(P, ffn_pool, pfx + "w3", BF16, tag="ww3", bufs=1),
                            b3=load_col(P, cols_pool, pfx + "b3"),
                            w4=load_w(P, ffn_pool, pfx + "w4", BF16, tag="ww4", bufs=1),
                            b4=load_row(P, cols_pool, pfx + "b4", F32R),
                        )
                        emit_ffn(P, fw, xcur, yy)
                    emit_ln(P, yy, load_col(P, cols_pool, pf + nm + "_g"),
                            load_col(P, cols_pool, pf + nm + "_b"), xcur)
                    ph.release()

        with tc.tile_pool(name="head", bufs=1) as hp:
            P.pools["ln"] = hp
            ones_row = P.consts["ones_row"]

            def relu6_evac(dst_t, bias):
                def f(ps, m, j):
                    t6 = hp.tile([NC, SCH], BF16, tag="ht6", bufs=3)
                    nc.scalar.activation(t6[:, :], ps[:, :], AF.Relu,
                                         bias=bias[:, m:m + 1])
                    nc.vector.tensor_scalar(dst_t[:, m, j * SCH:(j + 1) * SCH],
                                                t6[:, :], 6.0, None, op0=OP.min)
                return f

            f1 = hp.tile([NC, 4, S], BF16, tag="f1")
            proj_A(P, load_w(P, hp, "fu_w1", BF16, tag="wfu1", bufs=1), b_cur, 4, 4,
                   relu6_evac(f1, load_col(P, cols_pool, "fu_b1")), x2=l_cur, kt2=4)
            f2 = hp.tile([NC, 2, S], BF16, tag="f2")
            proj_A(P, load_w(P, hp, "fu_w2", BF16, tag="wfu2", bufs=1), f1, 4, 2,
                   relu6_evac(f2, load_col(P, cols_pool, "fu_b2")))
            f3 = hp.tile([NC, 4, S], BF16, tag="f3")
            fb3 = load_row(P, cols_pool, "fu_b3", F32R)

            def extra_f3(ps, m, j):
                nc.tensor.matmul(ps[:, :], fb3[:, m * NC:(m + 1) * NC],
                                 ones_row[:, 0:SCH], start=False, stop=True)

            def evac_f3(ps, m, j):
                nc.scalar.activation(f3[:, m, j * SCH:(j + 1) * SCH], ps[:, :], AF.Copy)

            proj_A(P, load_w(P, hp, "fu_w3", BF16, tag="wfu3", bufs=1), f2, 2, 4,
                   evac_f3, extra_mm=extra_f3)

            fo = hp.tile([NC, 4, S], BF16, tag="fo")
            emit_ln(P, f3, load_col(P, cols_pool, "fu_g"),
                    load_col(P, cols_pool, "fu_bb"), fo, relu=True)

            r1 = hp.tile([NC, 2, S], BF16, tag="r1")
            proj_A(P, load_w(P, hp, "rg_w1", BF16, tag="wrg1", bufs=1), fo, 4, 2,
                   relu6_evac(r1, load_col(P, cols_pool, "rg_b1")))
            r2 = hp.tile([NC, 1, S], BF16, tag="r2")
            proj_A(P, load_w(P, hp, "rg_w2", BF16, tag="wrg2", bufs=1), r1, 2, 1,
                   relu6_evac(r2, load_col(P, cols_pool, "rg_b2")))
            r3 = hp.tile([64, 1, S], BF16, tag="r3")
            rw3 = load_w(P, hp, "rg_w3", BF16, tag="wrg3", bufs=1)
            rb3 = load_col(P, cols_pool, "rg_b3")
            for j in range(P.NJ):
                ps = psum_t(P, [64, SCH])
                nc.tensor.matmul(ps[:, :], rw3[:, 0, 0:64],
                                 r2[:, 0, j * SCH:(j + 1) * SCH], start=True, stop=True)
                t6 = hp.tile([64, SCH], BF16, tag="r3t", bufs=3)
                nc.scalar.activation(t6[:, :], ps[:, :], AF.Relu, bias=rb3[0:64, 0:1])
                nc.vector.tensor_scalar(r3[:, 0, j * SCH:(j + 1) * SCH], t6[:, :],
                                        6.0, None, op0=OP.min)
            rw4 = load_w(P, hp, "rg_w4", BF16, tag="wrg4", bufs=1)
            rb4 = load_col(P, cols_pool, "rg_b4")
            outt = hp.tile([OUT, S], F32, tag="outt")
            for j in range(P.NJ):
                ps = psum_t(P, [OUT, SCH])
                nc.tensor.matmul(ps[:, :], rw4[:, 0, 0:OUT],
                                 r3[:, 0, j * SCH:(j + 1) * SCH], start=True, stop=True)
                nc.scalar.activation(outt[:, j * SCH:(j + 1) * SCH], ps[:, :],
                                     AF.Identity, bias=rb4[0:OUT, 0:1])
            nc.sync.dma_start(out_d.ap(), outt[:, :])

    P.nc.finalize()
    return P


def _run(body_feats, limb_feats, params, S, NL, trace=False, tmpdir=None):
    from concourse.bass_utils import run_bass_kernel_spmd

    B = body_feats.shape[0]
    packed = pack_params(params, NL)
    P = build_program(S, NL, packed)
    in_maps = []
    for c in range(B):
        m = {k: v for k, v in packed.items() if not k.startswith("_")}
        m["body"] = np.ascontiguousarray(
            np.asarray(body_feats[c], np.float32).T.reshape(4, NC, S)).astype(BF)
        m["limb"] = np.ascontiguousarray(
            np.asarray(limb_feats[c], np.float32).T.reshape(4, NC, S)).astype(BF)
        in_maps.append(m)
    res = run_bass_kernel_spmd(P.nc, in_maps, core_ids=list(range(B)),
                               trace=trace, tmpdir=tmpdir)
    outs = [np.asarray(res.results[c]["out"]).T for c in range(B)]
    return np.stack(outs, 0).astype(np.float32), res


def kernel(body_feats, limb_feats, params):
    out, _ = _run(body_feats, limb_feats, params, S=2048, NL=3)
    return out
